# revision 65
# baseline (speedup 1.0000x reference)
"""BinaryDilGroupConv Trainium2 kernel (v2).

Computes, for x[N=64, C=256, 32, 32]:
    h = BN(x)  (inference affine)
    a = sign(h); w = sign(weight)
    y = grouped dilated conv(a, w; groups=64, k=3, dil=2, pad=2)
    out = channel_shuffle(y, g=64) + x

Sharding: data-parallel over batch N across 8 NeuronCores (8 samples/core).
Params replicated. No collectives.

Device mapping (per core, per sample):
  - ACT: a = Sign(x*scale + bias) per 128-channel half, written fp8 into
    the interior of a zero-bordered padded tile (row pitch 40 bytes).
  - PE: grouped conv as block-diagonal matmuls. PSUM partition order is
    m = 32j + g for conv cout 4g + j (lhsT columns permuted on the
    host) so psum partition m of half h holds the value destined for
    final channel f = 64*(m//32) + 32h + (m%32). Dilation handled by
    shifted-window reads of the padded tile: the dy=0/dy=1 tap pairs run
    as fp8 DoubleRow matmuls with 4D strided rhs [p, 2, ny, 32] (pair
    stride 2 rows = 80B), the dy=2 taps as plain fp8 matmuls with 3D
    strided rhs [p, ny, 32] — both skip the pitch's junk columns, so
    PSUM tiles are dense 512-col (= one bank) per 16-row chunk.
  - Residual must be added in FINAL channel order (reference adds x
    after the shuffle), so the host sends a second, channel-permuted
    copy of x in fp16 (xf[m, h] = x[64*(m//32)+32h+(m%32)]; fp16
    rounding of the residual is ~5e-4 relative, far under the 2e-2
    gate, and halves the extra HBM traffic).
  - DVE: fused evict + residual: fin = psum + xf, fp16 out.
  - Stores are plain contiguous writes on the gpsimd DGE ring (device
    out layout [ns, 2, 128, S] fp16 is psum order; the host applies
    the inverse channel shuffle + fp32 cast on readback for free).
  - Engine/queue separation: loads=sync(q1), signs=scalar, matmuls=
    tensor, adds=vector, stores=gpsimd(q0); input pools fully buffered
    so no load ever WAR-waits behind a store or sign.
"""

import numpy as np
import ml_dtypes

C = 256
G = 64            # groups
CPG = 4           # channels per group
K = 3
DIL = 2
PAD = 2
EPS = 1e-5
H = W = 32
S = H * W         # 1024 spatial positions
PH = 38           # padded rows (36 used + 2 spill rows for flat windows)
PW = 40           # padded cols (36 used + 4: row pitch 40B makes the
                  # DoubleRow pair stride 80B, a multiple of 16)
N_FULL = 64
N_CORES = 8
NS = N_FULL // N_CORES   # samples per core
NHALF = 2                # channel halves of 128
CHUNKS = [(0, 16), (16, 16)]   # (y0, ny): ny*32 = 512 = one psum bank
ABUFS = 8                # padded-activation round-robin depth

_COMPILED = None


def build(n_samples=NS):
    """Build + compile the per-core Bass program."""
    import concourse.bass as bass
    import concourse.bacc as bacc
    import concourse.tile as tile
    import concourse.mybir as mybir

    fp32 = mybir.dt.float32
    fp16 = mybir.dt.float16
    fp8 = mybir.dt.float8e4

    nc = bacc.Bacc("TRN2", target_bir_lowering=False, debug=False,
                   num_devices=N_CORES)

    # partition-major layouts so load DMAs are contiguous 4KB runs
    xin = nc.dram_tensor("xin", [n_samples, 128, NHALF, S], fp32,
                         kind="ExternalInput").ap()
    # channel-permuted residual copy (fp16): xf[n, m, h] = x[n, f(m, h)]
    xfin = nc.dram_tensor("xfin", [n_samples, 128, NHALF, S], fp16,
                          kind="ExternalInput").ap()
    # weight free index = h*9 + dx*3 + slot (slot 0/1 = dy 0/1 pair
    # members, slot 2 = dy 2 single); columns in natural cout order
    wT = nc.dram_tensor("wT", [128, NHALF * K * K, 128], fp8,
                        kind="ExternalInput").ap()
    # pre-transposed on host: [128, 2] contiguous 8B/partition loads
    bnsc = nc.dram_tensor("bnsc", [128, NHALF], fp32,
                          kind="ExternalInput").ap()
    bnbi = nc.dram_tensor("bnbi", [128, NHALF], fp32,
                          kind="ExternalInput").ap()
    # row (h, m) holds final channel 64*(m//32) + 32h + (m%32); the
    # host un-permutes on readback. fp16 output (conv is small-integer
    # exact; fp16 rounds the fp32 sum at ~5e-4 relative) halves store
    # traffic — the DMA aggregate is the roofline here.
    out = nc.dram_tensor("out", [n_samples, NHALF, 128, S], fp16,
                         kind="ExternalOutput").ap()

    with tile.TileContext(nc) as tc:
        with (
            tc.tile_pool(name="const", bufs=1) as constp,
            tc.tile_pool(name="xp", bufs=2 * NS) as xp,
            tc.tile_pool(name="xfp", bufs=NS) as xfp,
            tc.tile_pool(name="finp", bufs=4) as finp,
            tc.tile_pool(name="psum", bufs=8, space="PSUM") as psump,
        ):
            # ---- prologue loads, all on the sync DGE ring in need-order:
            # BN params first (tiny), then x0h0, weights, x0h1, x1...
            sc_tile = constp.tile([128, NHALF], fp32)
            nc.sync.dma_start(sc_tile[:], bnsc)
            bi_tile = constp.tile([128, NHALF], fp32)
            nc.sync.dma_start(bi_tile[:], bnbi)

            x_nats = {}
            x_fs = {}

            def load_x(n, h):
                t = xp.tile([128, S], fp32, name=f"x_{h}", tag=f"x_{h}")
                x_nats[(n, h)] = t
                nc.sync.dma_start(t[:], xin[n][:, h, :])

            def load_xf(n):
                t = xfp.tile([128, NHALF, S], fp16, name="xf", tag="xf")
                x_fs[n] = t
                nc.sync.dma_start(t[:], xfin[n])

            # sample 0 arrives in y-pieces so the first Sign/matmul can
            # start early; weights right after the first piece
            SPLIT = 20 * W   # rows 0..20 cover chunk0's dy=2 reach (17)
            x0_tiles = {}
            t = xp.tile([128, S], fp32, name="x_0", tag="x_0")
            x_nats[(0, 0)] = t
            x0_tiles[0] = t
            nc.sync.dma_start(t[:, 0:SPLIT], xin[0][:, 0, 0:SPLIT])
            w_tile = constp.tile([128, NHALF * K * K, 128], fp8)
            nc.sync.dma_start(w_tile[:], wT)
            t = xp.tile([128, S], fp32, name="x_1", tag="x_1")
            x_nats[(0, 1)] = t
            x0_tiles[1] = t
            nc.sync.dma_start(t[:, 0:SPLIT], xin[0][:, 1, 0:SPLIT])
            for h in range(NHALF):
                nc.sync.dma_start(x0_tiles[h][:, SPLIT:S],
                                  xin[0][:, h, SPLIT:S])
            for h in range(NHALF):
                load_x(1, h)
            load_xf(0)
            load_xf(1)

            # warmup: trigger the ACT table load early and keep the PE
            # busy (pstate ramp) until the real stream starts
            warm_sb = constp.tile([128, 480], fp8)
            nc.gpsimd.memset(warm_sb[:], 0.0)
            warm_w = constp.tile([128, 128], fp8)
            nc.gpsimd.memset(warm_w[:], 0.0)
            warm_act = constp.tile([128, 16], fp8)
            nc.scalar.activation(warm_act[:], warm_sb[:, 0:16],
                                 mybir.ActivationFunctionType.Sign)
            # first batch: free-running; second batch reads the real
            # weight tile so it executes right before the first real
            # matmul, bridging the PE pstate-ramp window
            for _ in range(4):
                wps = psump.tile([128, 480], fp32, name="ps", tag="ps")
                nc.tensor.matmul(wps[:], warm_w[:], warm_sb[:],
                                 start=True, stop=True)
            for _ in range(1):
                wps = psump.tile([128, 480], fp32, name="ps", tag="ps")
                nc.tensor.matmul(wps[:], w_tile[:, 0, :], warm_sb[:],
                                 start=True, stop=True)

            # ---- persistent padded activation tiles, borders zeroed once
            a_pads = [[constp.tile([128, PH * PW], fp8,
                                   name=f"apad{h}_{b}")
                       for b in range(ABUFS)] for h in range(NHALF)]
            for h in range(NHALF):
                for b in range(ABUFS):
                    ap3 = a_pads[h][b][:].rearrange("p (y x) -> p y x", x=PW)
                    nc.gpsimd.memset(ap3[:, 0:PAD, :], 0.0)
                    nc.gpsimd.memset(ap3[:, PAD + H:PH, :], 0.0)
                    nc.gpsimd.memset(ap3[:, PAD:PAD + H, 0:PAD], 0.0)
                    nc.gpsimd.memset(ap3[:, PAD:PAD + H, PAD + W:PW], 0.0)

            # remaining x loads (sync ring, in consumption order)
            for n in range(2, n_samples):
                for h in range(NHALF):
                    load_x(n, h)
                load_xf(n)

            def window3(apad, offset, ny):
                """Per-row window AP [128, ny, 32] (rows at pitch PW)."""
                base = apad[:, offset:offset + 1]
                ap = [list(apad[:].ap[0]), [PW, ny], [1, W]]
                return bass.AP(base.tensor, base.offset, ap)

            def window4(apad, offset, ny):
                """DoubleRow per-row window [128, 2, ny, 32]: pair dim
                strides 2 rows (80B), row dim at pitch PW, 32 cols."""
                base = apad[:, offset:offset + 1]
                ap = [list(apad[:].ap[0]), [2 * PW, 2], [PW, ny], [1, W]]
                return bass.AP(base.tensor, base.offset, ap)

            for n in range(n_samples):
                fin = finp.tile([128, NHALF, S], fp16, name="fin",
                                tag="fin")
                xf = x_fs.pop(n)
                for h in range(NHALF):
                    x_nat = x_nats.pop((n, h))

                    # ---- a = Sign(x*scale + bias), fp8, padded interior
                    # (sample 0 in two y-halves to chase the split load)
                    ap3 = a_pads[h][n % ABUFS][:].rearrange(
                        "p (y x) -> p y x", x=PW)
                    x3 = x_nat[:].rearrange("p (y x) -> p y x", x=W)
                    for (r0, r1) in ([(0, 20), (20, 32)] if n == 0
                                     else [(0, 32)]):
                        nc.scalar.activation(
                            ap3[:, PAD + r0:PAD + r1, PAD:PAD + W],
                            x3[:, r0:r1, :],
                            mybir.ActivationFunctionType.Sign,
                            bias=bi_tile[:, h:h + 1],
                            scale=sc_tile[:, h:h + 1],
                        )

                    # ---- conv: fp8 DoubleRow pairs + 3D-strided singles
                    # per chunk, fused evict+residual into fin
                    apad = a_pads[h][n % ABUFS]
                    for ci, (y0, ny) in enumerate(CHUNKS):
                        ps = psump.tile([128, ny * W], fp32, name="ps",
                                        tag="ps")
                        ps3 = ps[:].rearrange("p (y x) -> p y x", x=W)
                        for dx in range(K):
                            wi = h * K * K + dx * K
                            nc.tensor.matmul(
                                ps3[:],
                                w_tile[:, wi:wi + 2, :],
                                window4(apad, y0 * PW + DIL * dx, ny),
                                start=(dx == 0), stop=False,
                                perf_mode=mybir.MatmulPerfMode.DoubleRow,
                            )
                        for dx in range(K):
                            wi = h * K * K + dx * K + 2
                            nc.tensor.matmul(
                                ps3[:],
                                w_tile[:, wi, :],
                                window3(apad,
                                        (y0 + 2 * DIL) * PW + DIL * dx, ny),
                                start=False, stop=(dx == K - 1),
                            )
                        sl = slice(y0 * W, (y0 + ny) * W)
                        nc.vector.tensor_add(fin[:, h, sl], ps[:],
                                             xf[:, h, sl])

                # ---- stores: contiguous, on the gpsimd ring (idle after
                # the prologue, so a store's wait blocks nothing else).
                # Last sample split per chunk across three rings so the
                # final drain isn't serialized on descriptor issue.
                if n == n_samples - 1:
                    rings = [nc.gpsimd, nc.sync, nc.scalar, nc.gpsimd]
                    for h in range(NHALF):
                        for ci, (y0, ny) in enumerate(CHUNKS):
                            sl = slice(y0 * W, (y0 + ny) * W)
                            rings[2 * h + ci].dma_start(out[n][h][:, sl],
                                                        fin[:, h, sl])
                else:
                    for h in range(NHALF):
                        nc.gpsimd.dma_start(out[n][h], fin[:, h, :])

    nc.compile()
    return nc


def _host_prep(x, weight, gamma, beta, running_mean, running_var):
    """Precompute BN affine + block-diagonal signed weights."""
    inv = (gamma / np.sqrt(running_var + EPS)).astype(np.float32)
    bias = (beta - running_mean * inv).astype(np.float32)
    wsign = np.sign(weight).astype(np.float32)   # [256, 4, 3, 3]

    lhsT = np.zeros((NHALF, K * K, 128, 128), np.float32)
    # Column m of lhsT (-> PSUM partition m) holds cout co = 4*(m%32)+m//32
    # within the half, so PSUM partition order is m = 32j + g for conv
    # cout 4g + j (matches the store AP and xperm layout).
    m = np.arange(128)
    co = CPG * (m % 32) + m // 32
    gl = co // CPG
    for h in range(NHALF):
        for dy in range(K):
            for dx in range(K):
                # device tap index: dx*3 + dy (dy 0/1 = DoubleRow pair)
                t = dx * K + dy
                for kk in range(CPG):
                    lhsT[h, t, CPG * gl + kk, m] = wsign[128 * h + co, kk,
                                                         dy, dx]
    # device weight layout: [ci, (h,t), m], fp8, contiguous upload
    lhsT = np.ascontiguousarray(
        lhsT.astype(ml_dtypes.float8_e4m3)
        .transpose(2, 0, 1, 3)
        .reshape(128, NHALF * K * K, 128))
    # pre-transposed [128, 2] so the device load is contiguous
    sc = np.ascontiguousarray(inv.reshape(NHALF, 128).T)
    bi = np.ascontiguousarray(bias.reshape(NHALF, 128).T)
    return lhsT, sc, bi


def _get_compiled():
    global _COMPILED
    if _COMPILED is None:
        _COMPILED = build(NS)
    return _COMPILED


def make_in_maps(x, weight, gamma, beta, running_mean, running_var):
    lhsT, sc, bi = _host_prep(x, weight, gamma, beta, running_mean,
                              running_var)
    # [cores, ns, 2, 128, S] -> partition-major [cores, ns, 128, 2, S]
    xs = np.ascontiguousarray(
        x.astype(np.float32)
        .reshape(N_CORES, NS, NHALF, 128, S)
        .transpose(0, 1, 3, 2, 4))
    # channel-permuted residual copy: xf[.., m, h, :] = x[.., f(m, h), :]
    # with f = 64*(m//32) + 32h + (m%32) (fp16: ~5e-4 relative rounding)
    m = np.arange(128)
    fidx = (64 * (m[:, None] // 32) + 32 * np.arange(NHALF)[None, :]
            + (m[:, None] % 32))                       # [128, 2]
    xf = np.ascontiguousarray(
        x.astype(np.float16)
        .reshape(N_CORES, NS, C, S)[:, :, fidx, :])    # [cores,ns,128,2,S]
    return [
        {"xin": xs[i], "xfin": xf[i], "wT": lhsT, "bnsc": sc, "bnbi": bi}
        for i in range(N_CORES)
    ]


def kernel(x, weight, gamma, beta, running_mean, running_var):
    from concourse.bass_utils import run_bass_kernel_spmd

    nc = _get_compiled()
    in_maps = make_in_maps(np.asarray(x), np.asarray(weight),
                           np.asarray(gamma), np.asarray(beta),
                           np.asarray(running_mean), np.asarray(running_var))
    res = run_bass_kernel_spmd(nc, in_maps, list(range(N_CORES)))
    # device out [ns, 2, 128, S]: row (h, m) = final channel
    # 64*(m//32) + 32h + (m%32); apply the inverse gather on the host
    f = np.arange(C)
    hh = (f % 64) // 32
    mm = 32 * (f // 64) + (f % 32)
    outs = [res.results[i]["out"].astype(np.float32)[:, hh, mm, :]
            .reshape(NS, C, H, W) for i in range(N_CORES)]
    return np.concatenate(outs, axis=0)
